# revision 1
# baseline (speedup 1.0000x reference)
"""Trainium2 Bass kernel: causal multi-head attention (B=4, T=2048, D=1024, H=16).

Sharding: tensor-parallel over heads. Each of the 8 cores handles 2 heads
(a 128-wide slice of the head dimension): it computes q/k/v projections for
its heads, causal attention, and a partial output projection
y_partial = o_local @ wo_local^T.  The full output is the sum of the 8
partials (reduced on host).

Device dataflow (all "transposed" so softmax reductions land on the free axis
of the PV matmul):
  qT,kT = w_loc @ x^T            [128, B*T]   (PSUM accum over 8 k-subtiles)
  vT -> PE-transpose -> v_nat    [tk=128, 64] per (b,h,block), augmented with
                                 a ones column (so PV also produces row sums Z)
  S^T block = kT_blk^T-contract  [tk=128, tq<=512]  (K=64 per head)
  E = exp(S^T * scale)           (ACT), causal mask on diagonal blocks (DVE)
  PV: psum[65, tq] += [v|1]^T-contract E   -> rows 0:64 = o_un^T, row 64 = Z
  o^T = o_un^T * (1/Z)           (batched recip + partition_broadcast + DVE mul)
  y_chunk = o^T_chunk^T @ wo^T   [tq=128, 512] -> DMA to DRAM

Batches are software-pipelined (A(b+1) emitted before B(b)) so the PE and ACT
always have ready fill work during stage-B dependency stalls.
"""

import os
import numpy as np

import concourse.bass as bass
import concourse.bacc as bacc
import concourse.mybir as mybir
from concourse.tile import TileContext
from concourse.masks import make_identity
from contextlib import ExitStack

# Problem constants (hardcoded per contract)
B, T, D, H = 4, 2048, 1024, 16
HD = D // H            # 64 head dim
P = 128                # partitions
KO = D // P            # 8 contraction subtiles for projections
TQT = 512              # tq tile width
NBLK = T // P          # 16 tk blocks per batch
NHL = 2                # heads per core
TT = B * T             # 8192 tokens
NCORES = 8
SCALE = 1.0 / float(np.sqrt(np.float32(HD)))

F32 = mybir.dt.float32
# float32r: fp32 data, full-rate-ish PE. bfloat16: 1 cycle/row + FWL. Override via env.
MM_DT = getattr(mybir.dt, os.environ.get("BASS_MM_DT", "float32r"))
# dtype for the v-transpose path (PE transpose input/identity)
TR_DT = MM_DT if MM_DT == mybir.dt.bfloat16 else F32


def _mm(ap):
    return ap


def build_program():
    nc = bacc.Bacc("TRN2", target_bir_lowering=False, num_devices=NCORES)
    xt = nc.dram_tensor("xt", [D, TT], MM_DT, kind="ExternalInput")
    wq = nc.dram_tensor("wq", [D, P], MM_DT, kind="ExternalInput")
    wk = nc.dram_tensor("wk", [D, P], MM_DT, kind="ExternalInput")
    wv = nc.dram_tensor("wv", [D, P], MM_DT, kind="ExternalInput")
    wo = nc.dram_tensor("wo", [P, D], MM_DT, kind="ExternalInput")
    cm = nc.dram_tensor("cmask", [4, P, TQT], F32, kind="ExternalInput")
    y = nc.dram_tensor("y", [TT, D], F32, kind="ExternalOutput")

    xt_r = xt[:].rearrange("(ko p) t -> p ko t", p=P)
    y_r = y[:].rearrange("(tn p) c -> p tn c", p=P)

    Exp = mybir.ActivationFunctionType.Exp
    Mult = mybir.AluOpType.mult

    with TileContext(nc) as tc, ExitStack() as ctx:
        const = ctx.enter_context(tc.tile_pool(name="const", bufs=1))
        qk_pool = ctx.enter_context(tc.tile_pool(name="qk", bufs=2))
        va_pool = ctx.enter_context(tc.tile_pool(name="va", bufs=2))
        o_pool = ctx.enter_context(tc.tile_pool(name="o", bufs=2))
        xt_pool = ctx.enter_context(tc.tile_pool(name="xtp", bufs=3))
        vt_pool = ctx.enter_context(tc.tile_pool(name="vt", bufs=2))
        e_pool = ctx.enter_context(tc.tile_pool(name="e", bufs=3))
        z_pool = ctx.enter_context(tc.tile_pool(name="z", bufs=3))
        y_pool = ctx.enter_context(tc.tile_pool(name="yp", bufs=2))
        psA = ctx.enter_context(tc.tile_pool(name="psA", bufs=2, space="PSUM"))
        psS = ctx.enter_context(tc.tile_pool(name="psS", bufs=2, space="PSUM"))
        psO = ctx.enter_context(tc.tile_pool(name="psO", bufs=2, space="PSUM"))

        # --- constants into SBUF ---
        wq_sb = const.tile([P, KO, P], MM_DT, tag="wq")
        wk_sb = const.tile([P, KO, P], MM_DT, tag="wk")
        wv_sb = const.tile([P, KO, P], MM_DT, tag="wv")
        for w_sb, w_d in ((wq_sb, wq), (wk_sb, wk), (wv_sb, wv)):
            nc.sync.dma_start(w_sb, w_d[:].rearrange("(ko p) d -> p ko d", p=P))
        wo_sb = const.tile([P, D], MM_DT, tag="wo")
        nc.sync.dma_start(wo_sb, wo[:])
        cm_sb = const.tile([P, 4, TQT], F32, tag="cm")
        nc.sync.dma_start(cm_sb, cm[:].rearrange("m p t -> p m t"))
        # identity for PE transposes, replicated on both head partition groups
        id_sb = const.tile([P, HD], TR_DT, tag="id")
        make_identity(nc, id_sb[0:HD, 0:HD])
        nc.vector.tensor_copy(id_sb[HD:P, :], id_sb[0:HD, :])

        def stage_a_alloc(b):
            """Allocate batch b's tile set and init constants."""
            qT = qk_pool.tile([P, T], MM_DT, tag="qT", name=f"qT{b}")
            kT = qk_pool.tile([P, T], MM_DT, tag="kT", name=f"kT{b}")
            va = va_pool.tile([P, NHL * NBLK, HD + 1], MM_DT, tag="va", name=f"va{b}")
            oT = o_pool.tile([P, T], MM_DT, tag="oT", name=f"oT{b}")
            zc = z_pool.tile([P, 2, TQT], F32, tag="zc", name=f"zc{b}")
            nc.vector.memset(zc, 1.0)
            nc.vector.tensor_copy(
                va[:, :, HD : HD + 1],
                nc.const_aps.tensor(1.0, (P, NHL * NBLK, 1), F32),
            )
            return qT, kT, va, oT, zc

        def stage_a_pair(b, tiles, pp):
            """Projections for two 512-token tiles of batch b, sharing each
            stationary weight chunk across both tiles' matmuls."""
            qT, kT, va, oT, zc = tiles
            tts = (2 * pp, 2 * pp + 1)
            xtl = []
            for s, tt in enumerate(tts):
                xx = xt_pool.tile([P, KO, TQT], MM_DT, tag="xt", name=f"xt{s}")
                nc.sync.dma_start(
                    xx, xt_r[:, :, b * T + tt * TQT : b * T + (tt + 1) * TQT]
                )
                xtl.append(xx)
            for w_sb, dst in ((wq_sb, qT), (wk_sb, kT)):
                pps = [
                    psA.tile([P, TQT], F32, tag="psA", name=f"pp{s}")
                    for s in range(2)
                ]
                for ko in range(KO):
                    for s in range(2):
                        nc.tensor.matmul(
                            pps[s],
                            _mm(w_sb[:, ko, :]),
                            _mm(xtl[s][:, ko, :]),
                            start=(ko == 0),
                            stop=(ko == KO - 1),
                        )
                for s, tt in enumerate(tts):
                    nc.vector.tensor_copy(
                        dst[:, tt * TQT : (tt + 1) * TQT], pps[s]
                    )
            for s, tt in enumerate(tts):
                psv = psA.tile([P, TQT], F32, tag="psA")
                for ko in range(KO):
                    nc.tensor.matmul(
                        psv,
                        _mm(wv_sb[:, ko, :]),
                        _mm(xtl[s][:, ko, :]),
                        start=(ko == 0),
                        stop=(ko == KO - 1),
                    )
                vt = vt_pool.tile([P, TQT], TR_DT, tag="vt")
                nc.vector.tensor_copy(vt, psv)
                # transpose v to natural layout [tk, d] per (head, block)
                for j in range(TQT // P):
                    blk = tt * (TQT // P) + j
                    for h in range(NHL):
                        hs = slice(h * HD, (h + 1) * HD)
                        pst = psA.tile([P, HD], TR_DT, tag="psA")
                        nc.tensor.transpose(
                            pst, vt[hs, j * P : (j + 1) * P], id_sb[hs, :]
                        )
                        nc.vector.tensor_copy(va[:, h * NBLK + blk, 0:HD], pst)

        def stage_b_qt(b, tiles, qt):
            """Attention for batch b. Both heads' S->exp->PV pipelines are
            interleaved per query tile so the PE never stalls on the exp
            round-trip latency."""
            qT, kT, va, oT, zc = tiles
            if True:
                tq0 = qt * TQT
                nblk = tq0 // P + TQT // P
                po = [
                    psO.tile([HD + 1, TQT], F32, tag="po", name=f"po{h}")
                    for h in range(NHL)
                ]
                for kb in range(nblk):
                    m = kb - tq0 // P  # >=0: diagonal-crossing block
                    c0 = P * m if m >= 0 else 0
                    ps2 = psS.tile([P, NHL, TQT], F32, tag="ps")
                    for h in range(NHL):
                        hs = slice(h * HD, (h + 1) * HD)
                        nc.tensor.matmul(
                            ps2[:, h, c0:TQT],
                            _mm(kT[hs, kb * P : (kb + 1) * P]),
                            _mm(qT[hs, tq0 + c0 : tq0 + TQT]),
                            start=True,
                            stop=True,
                        )
                    et2 = e_pool.tile([P, NHL, TQT], MM_DT, tag="et")
                    nc.scalar.activation(
                        et2[:, :, c0:TQT], ps2[:, :, c0:TQT], Exp, scale=SCALE
                    )
                    if m >= 0:
                        nc.vector.tensor_tensor(
                            et2[:, :, c0 : c0 + P],
                            et2[:, :, c0 : c0 + P],
                            cm_sb[:, m : m + 1, c0 : c0 + P].to_broadcast(
                                (P, NHL, P)
                            ),
                            Mult,
                        )
                    for h in range(NHL):
                        nc.tensor.matmul(
                            po[h][:, c0:TQT],
                            _mm(va[:, h * NBLK + kb, :]),
                            _mm(et2[:, h, c0:TQT]),
                            start=(kb == 0),
                            stop=(kb == nblk - 1),
                        )
                for h in range(NHL):
                    hs = slice(h * HD, (h + 1) * HD)
                    g = qt * NHL + h
                    gp = 32 * (g // 2)
                    nc.vector.tensor_copy(oT[hs, tq0 : tq0 + TQT], po[h][0:HD, :])
                    nc.vector.tensor_copy(
                        zc[gp : gp + 1, g % 2, :], po[h][HD : HD + 1, :]
                    )
        def stage_b_tail(b, tiles):
            """One wide reciprocal for all 8 (h, qt) groups, then broadcast
            1/Z across the head dim and normalize o^T in place."""
            qT, kT, va, oT, zc = tiles
            rc = z_pool.tile([P, 2, TQT], F32, tag="rc")
            nc.vector.reciprocal(rc, zc)
            for qt in range(T // TQT):
                for h in range(NHL):
                    hs = slice(h * HD, (h + 1) * HD)
                    g = qt * NHL + h
                    gp = 32 * (g // 2)
                    tq0 = qt * TQT
                    rr = z_pool.tile([1, TQT], F32, tag="rr", name=f"rr{g}")
                    nc.vector.tensor_copy(rr, rc[gp : gp + 1, g % 2, :])
                    rzb = z_pool.tile([P, TQT], F32, tag="rzb", name=f"rzb{g}")
                    nc.gpsimd.partition_broadcast(rzb, rr)
                    nc.vector.tensor_tensor(
                        oT[hs, tq0 : tq0 + TQT],
                        oT[hs, tq0 : tq0 + TQT],
                        rzb[hs, :],
                        Mult,
                    )

        def stage_c_part(b, tiles, part):
            """Quarter of the partial output projection for batch b."""
            qT, kT, va, oT, zc = tiles
            for tn in range(part * (T // P // 4), (part + 1) * (T // P // 4)):
                for cc in range(D // TQT):
                    psy = psA.tile([P, TQT], F32, tag="psA")
                    nc.tensor.matmul(
                        psy,
                        _mm(oT[:, tn * P : (tn + 1) * P]),
                        _mm(wo_sb[:, cc * TQT : (cc + 1) * TQT]),
                        start=True,
                        stop=True,
                    )
                    yt = y_pool.tile([P, TQT], F32, tag="yt")
                    nc.vector.tensor_copy(yt, psy)
                    nc.sync.dma_start(
                        y_r[:, b * (T // P) + tn, cc * TQT : (cc + 1) * TQT], yt
                    )

        # Fine-grained software pipeline: during batch b's attention, emit
        # next batch's projection tiles and previous batch's output projection
        # between query-tile groups so the PE always has dense fill work.
        def stage_a(b):
            t = stage_a_alloc(b)
            for pp in range(T // TQT // 2):
                stage_a_pair(b, t, pp)
            return t

        tiles = {0: stage_a(0)}
        for b in range(B):
            if b + 1 < B:
                tiles[b + 1] = stage_a(b + 1)
            for qt in range(T // TQT):
                stage_b_qt(b, tiles[b], qt)
            stage_b_tail(b, tiles[b])
            for part in range(4):
                stage_c_part(b, tiles[b], part)
            del tiles[b]

    nc.compile()
    return nc


def make_core_inputs(x, wq, wk, wv, wo):
    """Host-side sharding/layout prep. Returns list of 8 in_maps."""
    mdt = mybir.dt.np(MM_DT)
    x = np.ascontiguousarray(np.asarray(x, dtype=np.float32))
    wq = np.asarray(wq, dtype=np.float32).astype(mdt)
    wk = np.asarray(wk, dtype=np.float32).astype(mdt)
    wv = np.asarray(wv, dtype=np.float32).astype(mdt)
    wo = np.asarray(wo, dtype=np.float32).astype(mdt)

    xt = np.ascontiguousarray(x.reshape(TT, D).T).astype(mdt)  # [D, TT]
    # causal keep-masks for diagonal-crossing blocks, 4 shift variants
    i = np.arange(P)[:, None]
    j = np.arange(TQT)[None, :]
    cmask = np.stack(
        [(i + P * m <= j).astype(np.float32) for m in range(4)], axis=0
    )  # [4, P, TQT]

    in_maps = []
    for c in range(NCORES):
        dr = slice(c * P, (c + 1) * P)
        in_maps.append(
            {
                "xt": xt,
                "wq": np.ascontiguousarray(wq[dr, :].T),
                "wk": np.ascontiguousarray(wk[dr, :].T),
                "wv": np.ascontiguousarray(wv[dr, :].T),
                "wo": np.ascontiguousarray(wo[:, dr].T),
                "cmask": cmask,
            }
        )
    return in_maps


_CACHE = {}


def run(in_maps, **kwargs):
    from concourse.bass_utils import run_bass_kernel_spmd

    if "nc" not in _CACHE:
        _CACHE["nc"] = build_program()
    nc = _CACHE["nc"]
    res = run_bass_kernel_spmd(nc, in_maps, core_ids=list(range(NCORES)), **kwargs)
    return res


def kernel(x, wq, wk, wv, wo):
    in_maps = make_core_inputs(x, wq, wk, wv, wo)
    res = run(in_maps)
    y = np.zeros((TT, D), dtype=np.float32)
    for r in res.results:
        y += r["y"]
    return y.reshape(B, T, D)

